# revision 4
# baseline (speedup 1.0000x reference)
"""CapsuleLayer kernel for 8x Trainium2 NeuronCores.

Reference computes h = x @ W[0]  ([32,512]@[512,16384] -> [32,256,64] f32)
followed by 3 "routing" rounds:
    c = softmax(h, axis=1); h = einsum('bid,bjd->bjd', c, h)
The einsum contracts i only over c, so it equals h * sum_i c[b,i,d] = h * 1
(softmax sums to one over the contracted axis) -- the routing loop is the
identity up to f32 rounding (~1e-7 relative). The kernel therefore computes
just the matmul, sharded over the 16384-wide output dim across 8 cores so
each core streams a distinct 4 MiB slice of W (memory-bound roofline).

Numerics: x and W are split on the host into fp16 hi/lo pairs (exact to
~2^-22 relative) and the product (xh+xl)@(wh+wl) is computed with all four
terms at full fp16 PE rate with fp32 PSUM accumulation -- fp32-class
accuracy with the same 4 bytes/element of HBM traffic.

PE efficiency: the stationary operand is [xh | xl] (128x64), so one pass of
a w stream produces both the xh and xl partial products. The wh and wl
streams go to the two independent 128x64 column tiles of the PE array
(tile_position), so they process concurrently. The four partial blocks land
on disjoint PSUM partition ranges [4*32, NT]; the host sums the blocks.

Raw Bass (no TileContext) with a hand-rolled feed-forward pipeline: every
buffer is written exactly once, so the only semaphores are the natural
producer->consumer edges and there is no drain/barrier tail.
"""

import os

import numpy as np

B = 32          # batch
K = 512         # in_dim (contraction)
N_FULL = 16384  # num_capsules * out_dim
NUM_CAPS = 256
OUT_DIM = 64
NUM_CORES = 8
N_SHARD = N_FULL // NUM_CORES  # 2048 columns per core

KI = 128            # contraction partition tile
KO = K // KI        # 4 contraction subtiles
NT = 512            # output-column chunk (= max f32 PSUM bank free dim)
NCH = N_SHARD // NT  # 4 chunks per core
N_WARM = int(os.environ.get("CAPS_WARM", "8"))  # PE warmup matmuls (HAM ramp)

_NC = None
LAST_RESULTS = None  # BassKernelResults of the most recent run (for profiling)


def _build_nc():
    import concourse.bass as bass
    import concourse.mybir as mybir

    f16 = mybir.dt.float16
    f32 = mybir.dt.float32
    nc = bass.Bass("TRN2", target_bir_lowering=False)

    # Host-prepacked fp16 hi/lo pairs, contiguous per partition:
    #  xp[ki, ko, s, b]      = split(x)[s][b, ko*KI + ki]      (s = hi/lo)
    #  wp[j, ki, s, ko, t]   = split(W)[s][ko*KI + ki, n0 + j*NT + t]
    xp = nc.dram_tensor("xp", [KI, KO * 2 * B], f16, kind="ExternalInput")
    wp = nc.dram_tensor("wp", [NCH, KI, 2 * KO * NT], f16, kind="ExternalInput")
    # outd row blocks: [xh@wh | xl@wh | xh@wl | xl@wl], host sums them.
    outd = nc.dram_tensor("outd", [4 * B, N_SHARD], f32, kind="ExternalOutput")

    x_tile = nc.alloc_sbuf_tensor("x_tile", [KI, KO * 2 * B], f16)
    w_tiles = [
        nc.alloc_sbuf_tensor(f"w_tile{j}", [KI, 2 * KO * NT], f16)
        for j in range(NCH)
    ]
    o_tiles = [
        nc.alloc_sbuf_tensor(f"o_tile{j}", [4 * B, NT], f32) for j in range(NCH)
    ]
    warm_tile = nc.alloc_sbuf_tensor("warm_tile", [KI, NT], f16)

    ps_tiles = [nc.alloc_psum_tensor(f"ps{j}", [4 * B, NT], f32) for j in range(NCH)]
    ps_warm = nc.alloc_psum_tensor("ps_warm", [4 * B, NT], f32)

    x_ap = x_tile.ap().rearrange("ki (ko sb) -> ki ko sb", ko=KO)  # sb = 64
    w_aps = [
        w.ap().rearrange("ki (s ko t) -> ki s ko t", s=2, ko=KO) for w in w_tiles
    ]

    x_sem = nc.alloc_semaphore("x_sem")
    w_sem = nc.alloc_semaphore("w_sem")
    warm_sem = nc.alloc_semaphore("warm_sem")
    mm_sem = nc.alloc_semaphore("mm_sem")
    cp_sem = nc.alloc_semaphore("cp_sem")
    out_sem = nc.alloc_semaphore("out_sem")

    with nc.Block() as block:

        @block.gpsimd
        def _(gpsimd):
            gpsimd.memset(warm_tile[:], 0).then_inc(warm_sem, 1)
            gpsimd.dma_start(x_tile[:], xp[:]).then_inc(x_sem, 16)

        @block.sync
        def _(sync):
            for j in range(NCH):
                sync.dma_start(w_tiles[j][:], wp[j]).then_inc(w_sem, 16)

        @block.tensor
        def _(tensor):
            tensor.wait_ge(warm_sem, 1)
            for i in range(N_WARM):
                half = (i % 2) * 2 * B
                tensor.matmul(
                    ps_warm.ap()[half : half + 2 * B],
                    warm_tile[:, : 2 * B],
                    warm_tile[:],
                    start=True,
                    stop=True,
                )
            tensor.wait_ge(x_sem, 16)
            for j in range(NCH):
                tensor.wait_ge(w_sem, 16 * (j + 1))
                # column tile s (s=0: psum rows 0:64, s=1: rows 64:128)
                # runs the w_s stream; ko accumulates within each tile.
                for ko in range(KO):
                    for s in range(2):
                        ins = tensor.matmul(
                            ps_tiles[j].ap()[s * 2 * B : (s + 1) * 2 * B],
                            x_ap[:, ko, :],
                            w_aps[j][:, s, ko, :],
                            start=(ko == 0),
                            stop=(ko == KO - 1),
                        )
                ins.then_inc(mm_sem, 1)

        @block.vector
        def _(vector):
            for j in range(NCH):
                vector.wait_ge(mm_sem, j + 1)
                vector.tensor_copy(o_tiles[j][:], ps_tiles[j][:]).then_inc(cp_sem, 1)

        @block.scalar
        def _(scalar):
            for j in range(NCH):
                scalar.wait_ge(cp_sem, j + 1)
                scalar.dma_start(
                    outd[:, j * NT : (j + 1) * NT], o_tiles[j][:]
                ).then_inc(out_sem, 16)
            scalar.wait_ge(out_sem, 16 * NCH)

    return nc


def _get_nc():
    global _NC
    if _NC is None:
        _NC = _build_nc()
    return _NC


def _split_f16(a):
    hi = a.astype(np.float16)
    lo = (a - hi.astype(np.float32)).astype(np.float16)
    return hi, lo


def kernel(x, W):
    global LAST_RESULTS
    from concourse.bass_utils import run_bass_kernel_spmd

    x = np.ascontiguousarray(np.asarray(x, dtype=np.float32))
    W2 = np.ascontiguousarray(np.asarray(W, dtype=np.float32)).reshape(K, N_FULL)

    xh, xl = _split_f16(x)
    wh, wl = _split_f16(W2)

    # xp[ki, ko, s, b] = x_s[b, ko*KI + ki]  -> [KI, KO*2*B]
    xs = np.stack([xh, xl])  # [2, B, K]
    xp = np.ascontiguousarray(
        xs.transpose(2, 0, 1).reshape(KO, KI, 2, B).transpose(1, 0, 2, 3).reshape(
            KI, KO * 2 * B
        )
    )
    # wfull[jf, ki, s, ko, t] = w_s[ko*KI + ki, jf*NT + t]
    jf_total = N_FULL // NT
    ws = np.stack([wh, wl])  # [2, K, N]
    wfull = np.ascontiguousarray(
        ws.reshape(2, KO, KI, jf_total, NT).transpose(3, 2, 0, 1, 4).reshape(
            jf_total, KI, 2 * KO * NT
        )
    )

    nc = _get_nc()
    in_maps = []
    for c in range(NUM_CORES):
        in_maps.append({"xp": xp, "wp": wfull[c * NCH : (c + 1) * NCH]})

    res = run_bass_kernel_spmd(nc, in_maps, core_ids=list(range(NUM_CORES)))
    LAST_RESULTS = res
    # Sum the four partial row blocks, then stitch the core shards.
    parts = [r["outd"].reshape(4, B, N_SHARD).sum(axis=0) for r in res.results]
    full = np.concatenate(parts, axis=1)
    return full.reshape(B, NUM_CAPS, OUT_DIM).astype(np.float32)


# revision 8
# speedup vs baseline: 1.0642x; 1.0642x over previous
"""CapsuleLayer kernel for 8x Trainium2 NeuronCores.

Reference computes h = x @ W[0]  ([32,512]@[512,16384] -> [32,256,64] f32)
followed by 3 "routing" rounds:
    c = softmax(h, axis=1); h = einsum('bid,bjd->bjd', c, h)
The einsum contracts i only over c, so it equals h * sum_i c[b,i,d] = h * 1
(softmax sums to one over the contracted axis) -- the routing loop is the
identity up to f32 rounding (~1e-7 relative). The kernel therefore computes
just the matmul, sharded over the 16384-wide output dim across 8 cores so
each core streams a distinct 4 MiB slice of W (memory-bound roofline).

Numerics: x and W are split on the host into fp16 hi/lo pairs (exact to
~2^-22 relative) and the product (xh+xl)@(wh+wl) is computed with all four
terms at full fp16 PE rate with fp32 PSUM accumulation -- fp32-class
accuracy with the same 4 bytes/element of HBM traffic.

PE efficiency: the stationary operand is [xh | xl] (128x64), so one pass of
a w stream produces both the xh and xl partial products. The wh and wl
streams go to the two independent 128x64 column tiles of the PE array
(tile_position), so they process concurrently. The four partial blocks land
on disjoint PSUM partition ranges [4*32, NT]; the host sums the blocks.

Raw Bass (no TileContext) with a hand-rolled feed-forward pipeline: every
buffer is written exactly once, so the only semaphores are the natural
producer->consumer edges and there is no drain/barrier tail.
"""

import os

import numpy as np

B = 32          # batch
K = 512         # in_dim (contraction)
N_FULL = 16384  # num_capsules * out_dim
NUM_CAPS = 256
OUT_DIM = 64
NUM_CORES = 8
N_SHARD = N_FULL // NUM_CORES  # 2048 columns per core

KI = 128            # contraction partition tile
KO = K // KI        # 4 contraction subtiles
NT = int(os.environ.get("CAPS_NT", "256"))  # output-column chunk
NCH = N_SHARD // NT                          # chunks per core
N_WARM = int(os.environ.get("CAPS_WARM", "16"))  # PE warmup matmuls (HAM ramp)

_NC = None
LAST_RESULTS = None  # BassKernelResults of the most recent run (for profiling)


def _build_nc():
    import concourse.bass as bass
    import concourse.mybir as mybir

    f16 = mybir.dt.float16
    f32 = mybir.dt.float32
    nc = bass.Bass("TRN2", target_bir_lowering=False)

    # Host-prepacked fp16 hi/lo pairs, contiguous per partition:
    #  xp[ki, ko, s, b]      = split(x)[s][b, ko*KI + ki]      (s = hi/lo)
    #  wp[j, ki, s, ko, t]   = split(W)[s][ko*KI + ki, n0 + j*NT + t]
    xp = nc.dram_tensor("xp", [KI, KO * 2 * B], f16, kind="ExternalInput")
    wp = nc.dram_tensor("wp", [NCH, KI, 2 * KO * NT], f16, kind="ExternalInput")
    # outd row blocks: [xh@wh | xl@wh | xh@wl | xl@wl], host sums them.
    outd = nc.dram_tensor("outd", [4 * B, N_SHARD], f32, kind="ExternalOutput")

    x_tile = nc.alloc_sbuf_tensor("x_tile", [KI, KO * 2 * B], f16)
    w_tiles = [
        nc.alloc_sbuf_tensor(f"w_tile{j}", [KI, 2 * KO * NT], f16)
        for j in range(NCH)
    ]
    o_tiles = [
        nc.alloc_sbuf_tensor(f"o_tile{j}", [4 * B, NT], f32) for j in range(NCH)
    ]
    warm_tile = nc.alloc_sbuf_tensor("warm_tile", [KI, NT], f16)

    ps_tiles = [nc.alloc_psum_tensor(f"ps{j}", [4 * B, NT], f32) for j in range(NCH)]
    ps_warm = ps_tiles[0]  # warmup matmuls run before chunk 0; start=True resets

    x_ap = x_tile.ap().rearrange("ki (ko sb) -> ki ko sb", ko=KO)  # sb = 64
    w_aps = [
        w.ap().rearrange("ki (s ko t) -> ki s ko t", s=2, ko=KO) for w in w_tiles
    ]

    x_sem = nc.alloc_semaphore("x_sem")
    # One sem per W chunk: a shared counter is racy because each DMA's 16
    # SDMA engines increment independently and can skew across chunks.
    w_sems = [nc.alloc_semaphore(f"w_sem{j}") for j in range(NCH)]
    warm_sem = nc.alloc_semaphore("warm_sem")
    mm_sem = nc.alloc_semaphore("mm_sem")
    cp_sem = nc.alloc_semaphore("cp_sem")
    out_sem = nc.alloc_semaphore("out_sem")

    with nc.Block() as block:

        @block.gpsimd
        def _(gpsimd):
            gpsimd.memset(warm_tile[:], 0).then_inc(warm_sem, 1)
            gpsimd.dma_start(x_tile[:], xp[:]).then_inc(x_sem, 16)

        @block.sync
        def _(sync):
            for j in range(NCH):
                sync.dma_start(w_tiles[j][:], wp[j]).then_inc(w_sems[j], 16)

        @block.tensor
        def _(tensor):
            tensor.wait_ge(warm_sem, 1)
            for i in range(N_WARM):
                half = (i % 2) * 2 * B
                tensor.matmul(
                    ps_warm.ap()[half : half + 2 * B],
                    warm_tile[:, : 2 * B],
                    warm_tile[:],
                    start=True,
                    stop=True,
                )
            tensor.wait_ge(x_sem, 16)
            for j in range(NCH):
                tensor.wait_ge(w_sems[j], 16)
                # column tile s (s=0: psum rows 0:64, s=1: rows 64:128)
                # runs the w_s stream; ko accumulates within each tile.
                # The tiles complete independently, so the copy must wait on
                # BOTH tiles' final matmuls (2 incs per chunk).
                for ko in range(KO):
                    for s in range(2):
                        ins = tensor.matmul(
                            ps_tiles[j].ap()[s * 2 * B : (s + 1) * 2 * B],
                            x_ap[:, ko, :],
                            w_aps[j][:, s, ko, :],
                            start=(ko == 0),
                            stop=(ko == KO - 1),
                        )
                        if ko == KO - 1:
                            ins.then_inc(mm_sem, 1)

        @block.vector
        def _(vector):
            for j in range(NCH):
                vector.wait_ge(mm_sem, 2 * (j + 1))
                vector.tensor_copy(o_tiles[j][:], ps_tiles[j][:]).then_inc(cp_sem, 1)

        @block.scalar
        def _(scalar):
            for j in range(NCH):
                scalar.wait_ge(cp_sem, j + 1)
                scalar.dma_start(
                    outd[:, j * NT : (j + 1) * NT], o_tiles[j][:]
                ).then_inc(out_sem, 16)
            scalar.wait_ge(out_sem, 16 * NCH)

    return nc


def _get_nc():
    global _NC
    if _NC is None:
        _NC = _build_nc()
    return _NC


def _split_f16(a):
    hi = a.astype(np.float16)
    lo = (a - hi.astype(np.float32)).astype(np.float16)
    return hi, lo


def kernel(x, W):
    global LAST_RESULTS
    from concourse.bass_utils import run_bass_kernel_spmd

    x = np.ascontiguousarray(np.asarray(x, dtype=np.float32))
    W2 = np.ascontiguousarray(np.asarray(W, dtype=np.float32)).reshape(K, N_FULL)

    xh, xl = _split_f16(x)
    wh, wl = _split_f16(W2)

    # xp[ki, ko, s, b] = x_s[b, ko*KI + ki]  -> [KI, KO*2*B]
    xs = np.stack([xh, xl])  # [2, B, K]
    xp = np.ascontiguousarray(
        xs.transpose(2, 0, 1).reshape(KO, KI, 2, B).transpose(1, 0, 2, 3).reshape(
            KI, KO * 2 * B
        )
    )
    # wfull[jf, ki, s, ko, t] = w_s[ko*KI + ki, jf*NT + t]
    jf_total = N_FULL // NT
    ws = np.stack([wh, wl])  # [2, K, N]
    wfull = np.ascontiguousarray(
        ws.reshape(2, KO, KI, jf_total, NT).transpose(3, 2, 0, 1, 4).reshape(
            jf_total, KI, 2 * KO * NT
        )
    )

    nc = _get_nc()
    in_maps = []
    for c in range(NUM_CORES):
        in_maps.append({"xp": xp, "wp": wfull[c * NCH : (c + 1) * NCH]})

    res = run_bass_kernel_spmd(nc, in_maps, core_ids=list(range(NUM_CORES)))
    LAST_RESULTS = res
    # Sum the four partial row blocks, then stitch the core shards.
    parts = [r["outd"].reshape(4, B, N_SHARD).sum(axis=0) for r in res.results]
    full = np.concatenate(parts, axis=1)
    return full.reshape(B, NUM_CAPS, OUT_DIM).astype(np.float32)


# revision 16
# speedup vs baseline: 1.1333x; 1.0649x over previous
"""CapsuleLayer kernel for 8x Trainium2 NeuronCores.

Reference computes h = x @ W[0]  ([32,512]@[512,16384] -> [32,256,64] f32)
followed by 3 "routing" rounds:
    c = softmax(h, axis=1); h = einsum('bid,bjd->bjd', c, h)
The einsum contracts i only over c, so it equals h * sum_i c[b,i,d] = h * 1
(softmax sums to one over the contracted axis) -- the routing loop is the
identity up to f32 rounding (~1e-7 relative). The kernel therefore computes
just the matmul, sharded over the 16384-wide output dim across 8 cores so
each core streams a distinct 4 MiB slice of W (memory-bound roofline).

Numerics: x and W are split on the host into fp16 hi/lo pairs (exact to
~2^-22 relative) and the product (xh+xl)@(wh+wl) is computed with all four
terms at full fp16 PE rate with fp32 PSUM accumulation -- fp32-class
accuracy with the same 4 bytes/element of HBM traffic.

PE efficiency: the stationary operand is [xh | xl] (128x64), so one pass of
a w stream produces both the xh and xl partial products. The wh and wl
streams go to the two independent 128x64 column tiles of the PE array
(tile_position), so they process concurrently. The four partial blocks land
on disjoint PSUM partition ranges [4*32, NT]; the host sums the blocks.

Raw Bass (no TileContext) with a hand-rolled feed-forward pipeline: every
buffer is written exactly once, so the only semaphores are the natural
producer->consumer edges and there is no drain/barrier tail.
"""

import os

import numpy as np

B = 32          # batch
K = 512         # in_dim (contraction)
N_FULL = 16384  # num_capsules * out_dim
NUM_CAPS = 256
OUT_DIM = 64
NUM_CORES = 8
N_SHARD = N_FULL // NUM_CORES  # 2048 columns per core

KI = 128            # contraction partition tile
KO = K // KI        # 4 contraction subtiles
NT = int(os.environ.get("CAPS_NT", "256"))  # output-column chunk
NCH = N_SHARD // NT                          # chunks per core
N_WARM = int(os.environ.get("CAPS_WARM", "16"))  # PE warmup matmuls (HAM ramp)

_NC = None
LAST_RESULTS = None  # BassKernelResults of the most recent run (for profiling)


def _build_nc():
    import concourse.bass as bass
    import concourse.mybir as mybir

    f16 = mybir.dt.float16
    f32 = mybir.dt.float32
    nc = bass.Bass("TRN2", target_bir_lowering=False)

    # Host-prepacked fp16 hi/lo pairs, contiguous per partition:
    #  xp[ki, ko, s, b]      = split(x)[s][b, ko*KI + ki]      (s = hi/lo)
    #  wp[j, ki, s, ko, t]   = split(W)[s][ko*KI + ki, n0 + j*NT + t]
    bf16 = mybir.dt.bfloat16
    xp = nc.dram_tensor("xp", [KI, KO * 2 * B], f16, kind="ExternalInput")
    wp = nc.dram_tensor("wp", [NCH, KI, 2 * KO * NT], f16, kind="ExternalInput")
    # PSUM row blocks: [xh@wh | xl@wh | xh@wl | xl@wl]; the host sums them.
    # Block 0 leaves in f32; blocks 1+2 are ~2^-11 of the result so bf16
    # storage costs ~1e-6 relative; block 3 (~2^-22) is dropped entirely.
    out_hi = nc.dram_tensor("out_hi", [B, N_SHARD], f32, kind="ExternalOutput")
    out_lo = nc.dram_tensor("out_lo", [2 * B, N_SHARD], bf16, kind="ExternalOutput")

    x_tile = nc.alloc_sbuf_tensor("x_tile", [KI, KO * 2 * B], f16)
    w_tiles = [
        nc.alloc_sbuf_tensor(f"w_tile{j}", [KI, 2 * KO * NT], f16)
        for j in range(NCH)
    ]
    oh_tiles = [nc.alloc_sbuf_tensor(f"oh_tile{j}", [B, NT], f32) for j in range(NCH)]
    # DVE is lane-locked, so the lo tiles sit on partitions 32:96 to match
    # the PSUM blocks they copy from (rows 0:32 are unused padding).
    ol_tiles = [
        nc.alloc_sbuf_tensor(f"ol_tile{j}", [3 * B, NT], bf16) for j in range(NCH)
    ]
    warm_tile = nc.alloc_sbuf_tensor("warm_tile", [KI, NT], f16)

    ps_tiles = [nc.alloc_psum_tensor(f"ps{j}", [4 * B, NT], f32) for j in range(NCH)]
    ps_warm = ps_tiles[0]  # warmup matmuls run before chunk 0; start=True resets

    x_ap = x_tile.ap().rearrange("ki (ko sb) -> ki ko sb", ko=KO)  # sb = 64
    w_aps = [
        w.ap().rearrange("ki (s ko t) -> ki s ko t", s=2, ko=KO) for w in w_tiles
    ]

    x_sem = nc.alloc_semaphore("x_sem")
    # One sem per W chunk: a shared counter is racy because each DMA's 16
    # SDMA engines increment independently and can skew across chunks.
    w_sems = [nc.alloc_semaphore(f"w_sem{j}") for j in range(NCH)]
    warm_sem = nc.alloc_semaphore("warm_sem")
    mm_sem = nc.alloc_semaphore("mm_sem")
    cph_sem = nc.alloc_semaphore("cph_sem")
    cpl_sem = nc.alloc_semaphore("cpl_sem")
    outh_sem = nc.alloc_semaphore("outh_sem")
    outl_sem = nc.alloc_semaphore("outl_sem")

    with nc.Block() as block:

        @block.gpsimd
        def _(gpsimd):
            gpsimd.memset(warm_tile[:], 0).then_inc(warm_sem, 1)
            gpsimd.dma_start(x_tile[:], xp[:]).then_inc(x_sem, 16)

        @block.sync
        def _(sync):
            for j in range(NCH):
                sync.dma_start(w_tiles[j][:], wp[j]).then_inc(w_sems[j], 16)
            # Sync is free once the W loads are queued; it ships the bf16
            # lo blocks while Scalar ships the f32 hi blocks in parallel.
            for j in range(NCH):
                sync.wait_ge(cpl_sem, j + 1)
                sync.dma_start(
                    out_lo[:, j * NT : (j + 1) * NT], ol_tiles[j].ap()[B : 3 * B]
                ).then_inc(outl_sem, 16)
            sync.wait_ge(outl_sem, 16 * NCH)

        @block.tensor
        def _(tensor):
            tensor.wait_ge(warm_sem, 1)
            for i in range(N_WARM):
                half = (i % 2) * 2 * B
                tensor.matmul(
                    ps_warm.ap()[half : half + 2 * B],
                    warm_tile[:, : 2 * B],
                    warm_tile[:],
                    start=True,
                    stop=True,
                )
            tensor.wait_ge(x_sem, 16)
            for j in range(NCH):
                tensor.wait_ge(w_sems[j], 16)
                # column tile s (s=0: psum rows 0:64, s=1: rows 64:128)
                # runs the w_s stream; ko accumulates within each tile.
                # The tiles complete independently, so the copy must wait on
                # BOTH tiles' final matmuls (2 incs per chunk).
                for ko in range(KO):
                    for s in range(2):
                        ins = tensor.matmul(
                            ps_tiles[j].ap()[s * 2 * B : (s + 1) * 2 * B],
                            x_ap[:, ko, :],
                            w_aps[j][:, s, ko, :],
                            start=(ko == 0),
                            stop=(ko == KO - 1),
                        )
                        if ko == KO - 1:
                            ins.then_inc(mm_sem, 1)

        @block.vector
        def _(vector):
            for j in range(NCH):
                vector.wait_ge(mm_sem, 2 * (j + 1))
                vector.tensor_copy(oh_tiles[j][:], ps_tiles[j].ap()[:B]).then_inc(
                    cph_sem, 1
                )
                # PSUM access patterns may span at most 32 partitions when
                # starting at partition 32 -> two lane-aligned copies.
                vector.tensor_copy(
                    ol_tiles[j].ap()[B : 2 * B], ps_tiles[j].ap()[B : 2 * B]
                )
                vector.tensor_copy(
                    ol_tiles[j].ap()[2 * B : 3 * B], ps_tiles[j].ap()[2 * B : 3 * B]
                ).then_inc(cpl_sem, 1)

        @block.scalar
        def _(scalar):
            for j in range(NCH):
                scalar.wait_ge(cph_sem, j + 1)
                scalar.dma_start(
                    out_hi[:, j * NT : (j + 1) * NT], oh_tiles[j][:]
                ).then_inc(outh_sem, 16)
            scalar.wait_ge(outh_sem, 16 * NCH)

    return nc


def _get_nc():
    global _NC
    if _NC is None:
        _NC = _build_nc()
    return _NC


def _split_f16(a):
    hi = a.astype(np.float16)
    lo = (a - hi.astype(np.float32)).astype(np.float16)
    return hi, lo


def kernel(x, W):
    global LAST_RESULTS
    from concourse.bass_utils import run_bass_kernel_spmd

    x = np.ascontiguousarray(np.asarray(x, dtype=np.float32))
    W2 = np.ascontiguousarray(np.asarray(W, dtype=np.float32)).reshape(K, N_FULL)

    xh, xl = _split_f16(x)
    wh, wl = _split_f16(W2)

    # xp[ki, ko, s, b] = x_s[b, ko*KI + ki]  -> [KI, KO*2*B]
    xs = np.stack([xh, xl])  # [2, B, K]
    xp = np.ascontiguousarray(
        xs.transpose(2, 0, 1).reshape(KO, KI, 2, B).transpose(1, 0, 2, 3).reshape(
            KI, KO * 2 * B
        )
    )
    # wfull[jf, ki, s, ko, t] = w_s[ko*KI + ki, jf*NT + t]
    jf_total = N_FULL // NT
    ws = np.stack([wh, wl])  # [2, K, N]
    wfull = np.ascontiguousarray(
        ws.reshape(2, KO, KI, jf_total, NT).transpose(3, 2, 0, 1, 4).reshape(
            jf_total, KI, 2 * KO * NT
        )
    )

    nc = _get_nc()
    in_maps = []
    for c in range(NUM_CORES):
        in_maps.append({"xp": xp, "wp": wfull[c * NCH : (c + 1) * NCH]})

    res = run_bass_kernel_spmd(nc, in_maps, core_ids=list(range(NUM_CORES)))
    LAST_RESULTS = res
    # out = hi block + the two bf16 cross-term blocks, stitched across cores.
    parts = [
        r["out_hi"]
        + r["out_lo"].astype(np.float32).reshape(2, B, N_SHARD).sum(axis=0)
        for r in res.results
    ]
    full = np.concatenate(parts, axis=1)
    return full.reshape(B, NUM_CAPS, OUT_DIM).astype(np.float32)


# revision 17
# speedup vs baseline: 1.1390x; 1.0050x over previous
"""CapsuleLayer kernel for 8x Trainium2 NeuronCores.

Reference computes h = x @ W[0]  ([32,512]@[512,16384] -> [32,256,64] f32)
followed by 3 "routing" rounds:
    c = softmax(h, axis=1); h = einsum('bid,bjd->bjd', c, h)
The einsum contracts i only over c, so it equals h * sum_i c[b,i,d] = h * 1
(softmax sums to one over the contracted axis) -- the routing loop is the
identity up to f32 rounding (~1e-7 relative). The kernel therefore computes
just the matmul, sharded over the 16384-wide output dim across 8 cores so
each core streams a distinct 4 MiB slice of W (memory-bound roofline).

Numerics: x and W are split on the host into fp16 hi/lo pairs (exact to
~2^-22 relative) and (xh+xl)@(wh+wl) is computed at full fp16 PE rate with
fp32 PSUM accumulation -- fp32-class accuracy with the same 4 bytes/element
of HBM traffic.

PE efficiency: the stationary operand is [xh | xl] (128x64), so one pass of
a w stream produces both the xh and xl partial products, and the wh / wl
streams run concurrently on the two independent 128x64 column tiles of the
PE array. The partial blocks [xh@wh | xl@wh | xh@wl | xl@wl] land on
disjoint 32-partition PSUM ranges; block 0 leaves in f32, blocks 1+2
(~2^-11 of the result) leave in bf16, block 3 (~2^-22) is dropped, and the
host sums the blocks.

Raw Bass (no TileContext) with a hand-rolled feed-forward pipeline: every
buffer is written exactly once, so the only semaphores are the natural
producer->consumer edges. W streams in column chunks; the final chunks are
small so the end-of-kernel receipt->compute->copy->writeback chain is short.
"""

import os

import numpy as np

B = 32          # batch
K = 512         # in_dim (contraction)
N_FULL = 16384  # num_capsules * out_dim
NUM_CAPS = 256
OUT_DIM = 64
NUM_CORES = 8
N_SHARD = N_FULL // NUM_CORES  # 2048 columns per core

KI = 128            # contraction partition tile
KO = K // KI        # 4 contraction subtiles
# Column-chunk widths per core (sum = N_SHARD). Uniform while streaming,
# tapering at the end to shorten the kernel tail.
CHUNKS = [256] * 7 + [192, 64]
assert sum(CHUNKS) == N_SHARD
NCH = len(CHUNKS)
OFFS = [sum(CHUNKS[:i]) for i in range(NCH)]
N_PSUM = 8          # PSUM banks; chunks beyond 8 reuse bank (j % N_PSUM)
N_WARM = int(os.environ.get("CAPS_WARM", "16"))  # PE warmup matmuls (HAM ramp)

_NC = None
LAST_RESULTS = None  # BassKernelResults of the most recent run (for profiling)


def _build_nc():
    import concourse.bass as bass
    import concourse.mybir as mybir

    f16 = mybir.dt.float16
    f32 = mybir.dt.float32
    bf16 = mybir.dt.bfloat16
    nc = bass.Bass("TRN2", target_bir_lowering=False)

    # Host-prepacked fp16 hi/lo pairs, contiguous per partition:
    #  xp[ki, ko, s, b]  = split(x)[s][b, ko*KI + ki]          (s = hi/lo)
    #  wp[ki, chunk-major: (s, ko, t)] = split(W)[s][ko*KI + ki, n0 + off_j + t]
    xp = nc.dram_tensor("xp", [KI, KO * 2 * B], f16, kind="ExternalInput")
    wp = nc.dram_tensor("wp", [KI, 2 * KO * N_SHARD], f16, kind="ExternalInput")
    out_hi = nc.dram_tensor("out_hi", [B, N_SHARD], f32, kind="ExternalOutput")
    out_lo = nc.dram_tensor("out_lo", [2 * B, N_SHARD], bf16, kind="ExternalOutput")

    x_tile = nc.alloc_sbuf_tensor("x_tile", [KI, KO * 2 * B], f16)
    w_tiles = [
        nc.alloc_sbuf_tensor(f"w_tile{j}", [KI, 2 * KO * CHUNKS[j]], f16)
        for j in range(NCH)
    ]
    oh_tiles = [
        nc.alloc_sbuf_tensor(f"oh_tile{j}", [B, CHUNKS[j]], f32) for j in range(NCH)
    ]
    # DVE is lane-locked, so the lo tiles sit on partitions 32:96 to match
    # the PSUM blocks they copy from (rows 0:32 are unused padding).
    ol_tiles = [
        nc.alloc_sbuf_tensor(f"ol_tile{j}", [3 * B, CHUNKS[j]], bf16)
        for j in range(NCH)
    ]
    warm_tile = nc.alloc_sbuf_tensor("warm_tile", [KI, 256], f16)

    NT_MAX = max(CHUNKS)
    ps_tiles = [
        nc.alloc_psum_tensor(f"ps{p}", [4 * B, NT_MAX], f32) for p in range(N_PSUM)
    ]
    ps_warm = ps_tiles[0]  # warmup matmuls run before chunk 0; start=True resets

    x_ap = x_tile.ap().rearrange("ki (ko sb) -> ki ko sb", ko=KO)  # sb = 64
    w_aps = [
        w.ap().rearrange("ki (s ko t) -> ki s ko t", s=2, ko=KO) for w in w_tiles
    ]

    x_sem = nc.alloc_semaphore("x_sem")
    # One sem per W chunk: a shared counter is racy because each DMA's 16
    # SDMA engines increment independently and can skew across chunks.
    w_sems = [nc.alloc_semaphore(f"w_sem{j}") for j in range(NCH)]
    warm_sem = nc.alloc_semaphore("warm_sem")
    mm_sem = nc.alloc_semaphore("mm_sem")
    cph_sem = nc.alloc_semaphore("cph_sem")
    cpl_sem = nc.alloc_semaphore("cpl_sem")
    outh_sem = nc.alloc_semaphore("outh_sem")
    outl_sem = nc.alloc_semaphore("outl_sem")

    with nc.Block() as block:

        @block.gpsimd
        def _(gpsimd):
            gpsimd.memset(warm_tile[:], 0).then_inc(warm_sem, 1)
            gpsimd.dma_start(x_tile[:], xp[:]).then_inc(x_sem, 16)

        @block.sync
        def _(sync):
            for j in range(NCH):
                sync.dma_start(
                    w_tiles[j][:],
                    wp[:, 2 * KO * OFFS[j] : 2 * KO * (OFFS[j] + CHUNKS[j])],
                ).then_inc(w_sems[j], 16)
            # Sync is free once the W loads are queued; it ships the bf16
            # lo blocks while Scalar ships the f32 hi blocks in parallel.
            for j in range(NCH):
                sync.wait_ge(cpl_sem, j + 1)
                sync.dma_start(
                    out_lo[:, OFFS[j] : OFFS[j] + CHUNKS[j]],
                    ol_tiles[j].ap()[B : 3 * B],
                ).then_inc(outl_sem, 16)
            sync.wait_ge(outl_sem, 16 * NCH)

        @block.tensor
        def _(tensor):
            tensor.wait_ge(warm_sem, 1)
            for i in range(N_WARM):
                half = (i % 2) * 2 * B
                tensor.matmul(
                    ps_warm.ap()[half : half + 2 * B, :256],
                    warm_tile[:, : 2 * B],
                    warm_tile[:],
                    start=True,
                    stop=True,
                )
            tensor.wait_ge(x_sem, 16)
            for j in range(NCH):
                tensor.wait_ge(w_sems[j], 16)
                if j >= N_PSUM:
                    # Bank reuse: the copies of chunk j - N_PSUM must be done.
                    tensor.wait_ge(cph_sem, j - N_PSUM + 1)
                    tensor.wait_ge(cpl_sem, j - N_PSUM + 1)
                ps = ps_tiles[j % N_PSUM]
                # Column tile s (s=0: psum rows 0:64, s=1: rows 64:128) runs
                # the w_s stream; ko accumulates within each tile. The tiles
                # complete independently, so the copy waits on BOTH tiles'
                # final matmuls (2 incs per chunk).
                for ko in range(KO):
                    for s in range(2):
                        ins = tensor.matmul(
                            ps.ap()[s * 2 * B : (s + 1) * 2 * B, : CHUNKS[j]],
                            x_ap[:, ko, :],
                            w_aps[j][:, s, ko, :],
                            start=(ko == 0),
                            stop=(ko == KO - 1),
                        )
                        if ko == KO - 1:
                            ins.then_inc(mm_sem, 1)

        @block.vector
        def _(vector):
            for j in range(NCH):
                vector.wait_ge(mm_sem, 2 * (j + 1))
                ps = ps_tiles[j % N_PSUM]
                vector.tensor_copy(
                    oh_tiles[j][:], ps.ap()[:B, : CHUNKS[j]]
                ).then_inc(cph_sem, 1)
                # PSUM access patterns may span at most 32 partitions when
                # starting at partition 32 -> two lane-aligned copies.
                vector.tensor_copy(
                    ol_tiles[j].ap()[B : 2 * B], ps.ap()[B : 2 * B, : CHUNKS[j]]
                )
                vector.tensor_copy(
                    ol_tiles[j].ap()[2 * B : 3 * B],
                    ps.ap()[2 * B : 3 * B, : CHUNKS[j]],
                ).then_inc(cpl_sem, 1)

        @block.scalar
        def _(scalar):
            for j in range(NCH):
                scalar.wait_ge(cph_sem, j + 1)
                scalar.dma_start(
                    out_hi[:, OFFS[j] : OFFS[j] + CHUNKS[j]], oh_tiles[j][:]
                ).then_inc(outh_sem, 16)
            scalar.wait_ge(outh_sem, 16 * NCH)

    return nc


def _get_nc():
    global _NC
    if _NC is None:
        _NC = _build_nc()
    return _NC


def _split_f16(a):
    hi = a.astype(np.float16)
    lo = (a - hi.astype(np.float32)).astype(np.float16)
    return hi, lo


def kernel(x, W):
    global LAST_RESULTS
    from concourse.bass_utils import run_bass_kernel_spmd

    x = np.ascontiguousarray(np.asarray(x, dtype=np.float32))
    W2 = np.ascontiguousarray(np.asarray(W, dtype=np.float32)).reshape(K, N_FULL)

    xh, xl = _split_f16(x)
    wh, wl = _split_f16(W2)

    # xp[ki, ko, s, b] = x_s[b, ko*KI + ki]  -> [KI, KO*2*B]
    xs = np.stack([xh, xl])  # [2, B, K]
    xp = np.ascontiguousarray(
        xs.transpose(2, 0, 1).reshape(KO, KI, 2, B).transpose(1, 0, 2, 3).reshape(
            KI, KO * 2 * B
        )
    )
    # wk[ki, s, ko, n] = w_s[ko*KI + ki, n]  (full width, then chunk-sliced)
    ws = np.stack([wh, wl])  # [2, K, N]
    wk = ws.reshape(2, KO, KI, N_FULL).transpose(2, 0, 1, 3)  # [KI, 2, KO, N]

    nc = _get_nc()
    in_maps = []
    for c in range(NUM_CORES):
        n0 = c * N_SHARD
        # Chunk-major packing: per partition, chunk j's (s, ko, t) block is
        # contiguous so each chunk is a single contiguous-per-partition DMA.
        blocks = [
            wk[:, :, :, n0 + OFFS[j] : n0 + OFFS[j] + CHUNKS[j]].reshape(KI, -1)
            for j in range(NCH)
        ]
        wp = np.ascontiguousarray(np.concatenate(blocks, axis=1))
        in_maps.append({"xp": xp, "wp": wp})

    res = run_bass_kernel_spmd(nc, in_maps, core_ids=list(range(NUM_CORES)))
    LAST_RESULTS = res
    # out = hi block + the two bf16 cross-term blocks, stitched across cores.
    parts = [
        r["out_hi"]
        + r["out_lo"].astype(np.float32).reshape(2, B, N_SHARD).sum(axis=0)
        for r in res.results
    ]
    full = np.concatenate(parts, axis=1)
    return full.reshape(B, NUM_CAPS, OUT_DIM).astype(np.float32)
